# revision 21
# baseline (speedup 1.0000x reference)
"""Trainium2 Bass kernel for nn_Attention (B=4, N=2048, DIM=512, H=8).

Sharding: 8 cores = (batch b, seq-half s). Each core computes attention
outputs for queries [s*1024, (s+1)*1024) of batch b, all 8 heads, plus
the output projection for those rows. Outputs are disjoint -> host
gather is a pure concatenation (no reduction). Keys are permuted per
core (own seq-half first) so the query chunk is always columns [0, NQ)
of the permuted x.T; attention is permutation-invariant over keys.

Dataflow (matmul operands bf16, PSUM accumulation f32): the Scalar
engine's 128 exp tiles (~1.1us each) are the hard floor; everything is
scheduled around keeping the exp pipeline dense:

  - inputs land via 13 batched DMA triggers (priority-ordered; the
    trigger instruction itself costs ~620ns of engine time, so fewer +
    earlier triggers move the first exp from ~30us to ~8us)
  - PSUM: scores ring 2x[128,1024] (4 banks) fed only by score matmuls
    and drained only by exp; pv accumulators ring 2x[128,512] (2
    banks, both heads of a pair share the ring across iterations); a
    2x[128,512] staging ring (2 banks) for projection emits / the
    norm broadcast / phase3
  - per kt-pair: 2 score tiles -> 2 exps -> ONE [128,2048] mask-mul
    (praw * exp(mask), both kts at once, 2x DVE mode); one mul per
    iteration runs on GpSimd (SBUF-only op) to unload the DVE
  - p@v is issued with a 2-ktp lag so its phat operand is always
    ready (the PE queue is in-order; a blocked head-of-line matmul
    would stall independent work queued behind it)
  - QKV/proj projections are woven between score matmuls as
    independent PE filler; phase3 (out proj) is interleaved into the
    half-1 iterations so the tail after the last exp is ~1 block
  - normalization: softmax sums ride in PSUM row 64 (ones column in
    v), partition-broadcast via two K=1 PE matmuls, reciprocal + mul
    on DVE, deferred into the next iteration's first ktp slots
"""
import functools
import numpy as np
import ml_dtypes
from contextlib import ExitStack

import concourse.bass as bass
import concourse.tile as tile
from concourse import bacc, mybir
from concourse.bass_utils import run_bass_kernel_spmd

F32 = mybir.dt.float32
BF16 = mybir.dt.bfloat16
AF = mybir.ActivationFunctionType

B, N, DIM, H, D = 4, 2048, 512, 8, 64
SCALE = D ** -0.5
NQ = N // 2          # queries per core
NKT = N // 128       # key tiles (16)
NKTP = NKT // 2      # kt pairs per iteration (8)
NCORES = 8


def build(dbg=False):
    nc = bacc.Bacc("TRN2", target_bir_lowering=False, debug=False,
                   num_devices=NCORES)
    xT = nc.dram_tensor("xT", [DIM, N], BF16, kind="ExternalInput").ap()
    wqT = nc.dram_tensor("wqT", [DIM, DIM], BF16, kind="ExternalInput").ap()
    wkT = nc.dram_tensor("wkT", [DIM, DIM], BF16, kind="ExternalInput").ap()
    wvT = nc.dram_tensor("wvT", [DIM, DIM], BF16, kind="ExternalInput").ap()
    projT = nc.dram_tensor("projT", [DIM, DIM], BF16, kind="ExternalInput").ap()
    biasb = nc.dram_tensor("biasb", [128, DIM], F32, kind="ExternalInput").ap()
    expmT = nc.dram_tensor("expmT", [N, NQ], BF16, kind="ExternalInput").ap()
    out = nc.dram_tensor("out", [NQ, DIM], F32, kind="ExternalOutput").ap()

    with tile.TileContext(nc) as tc, ExitStack() as ctx:
        # ---- SBUF pools ----
        wp = ctx.enter_context(tc.tile_pool(name="wp", bufs=1))
        kv = ctx.enter_context(tc.tile_pool(name="kv", bufs=1))
        small = ctx.enter_context(tc.tile_pool(name="small", bufs=2))
        osb = ctx.enter_context(tc.tile_pool(name="osb", bufs=2))
        praw_p = ctx.enter_context(tc.tile_pool(name="praw", bufs=4))
        phat_p = ctx.enter_context(tc.tile_pool(name="phat", bufs=6))
        # ---- PSUM pools: 2x2 + 2x1 + 2x1 = 8 banks ----
        ps_score = ctx.enter_context(
            tc.tile_pool(name="ps_score", bufs=2, space="PSUM"))  # 2x2 banks
        ps_pv = ctx.enter_context(
            tc.tile_pool(name="ps_pv", bufs=2, space="PSUM"))     # 2x1 bank
        ps_proj = ctx.enter_context(
            tc.tile_pool(name="ps_proj", bufs=2, space="PSUM"))   # 2x1 bank

        # ---- persistent tiles (combined so one DMA fills each) ----
        wq_b = wp.tile([128, 4 * DIM], BF16, name="wq_b", tag="wq_b")
        wk_b = wp.tile([128, 4 * DIM], BF16, name="wk_b", tag="wk_b")
        wv_b = wp.tile([128, 4 * DIM], BF16, name="wv_b", tag="wv_b")
        pj_b = wp.tile([128, 4 * DIM], BF16, name="pj_b", tag="pj_b")
        x_b = wp.tile([128, 4 * N], BF16, name="x_b", tag="x_b")
        bias_sb = wp.tile([128, DIM], F32, name="bias_sb", tag="bias_sb")
        ones_row = wp.tile([1, 64], BF16, name="ones_row", tag="ones_row")
        warm = wp.tile([1, 2], F32, name="warm", tag="warm")
        # em groups: (g, qhalf) holds kt = 4g..4g+3 for one query half,
        # layout [128, (i, q512)] -- split by q-half so only 2MB of mask
        # is needed during the first four iterations
        em_g = [[wp.tile([128, 4 * 512], BF16, name=f"em{g}_{h}",
                         tag=f"em{g}_{h}") for h in range(2)]
                for g in range(4)]
        q_sb = [kv.tile([128, NQ], BF16, name=f"q{m}", tag=f"q{m}")
                for m in range(4)]
        k_sb = [kv.tile([128, N], BF16, name=f"k{m}", tag=f"k{m}")
                for m in range(4)]
        # per-head 65th column is ones -> sums row lands in PSUM row 64
        v_sb = [kv.tile([128, 8 * 65], BF16, name=f"v{kt}", tag=f"v{kt}")
                for kt in range(NKT)]
        uhat = [kv.tile([128, NQ], BF16, name=f"uh{p}", tag=f"uh{p}")
                for p in range(4)]

        # warm the activation table (implicit ACT_TABLE_LOAD happens on
        # the first Exp; issue it at t=0 so the ~1.3us load overlaps DMA)
        nc.gpsimd.memset(warm[:], 0.0)
        nc.scalar.activation(warm[:], warm[:], AF.Exp)

        nc.gpsimd.memset(ones_row[:], 1.0)
        for kt in range(NKT):
            nc.gpsimd.memset(
                v_sb[kt][:].rearrange("p (h c) -> p h c", h=8)[:, :, 64:65],
                1.0)

        # ---- input DMAs: batched (one trigger per tensor/chunk),
        #      priority-ordered by first use ----
        def folded(src, cols):
            return src[:, cols].rearrange("(kc p) j -> p kc j", kc=4)

        # Criticality order, interleaved across the three DGE queues so
        # the first-needed tensors drain in parallel:
        #   q(0,0) needs wq+x1a; k(0,0,0) needs wk; v(0) needs wv;
        #   em half-0 during it0-3; x2 by it0/ktp2; em half-1 from it4.
        x_v = x_b[:].rearrange("p (kc j) -> p kc j", kc=4)

        def em_dma(eng, g, h):
            src = expmT[g * 512:(g + 1) * 512, h * 512:(h + 1) * 512]
            eng.dma_start(
                em_g[g][h][:].rearrange("p (i q) -> p i q", i=4),
                src.rearrange("(i p) q -> p i q", i=4))

        wq_v = wq_b[:].rearrange("p (kc j) -> p kc j", kc=4)
        wk_v = wk_b[:].rearrange("p (kc j) -> p kc j", kc=4)
        # first m-block of wq/wk ships separately (128KB) so the
        # prologue emits unblock as early as possible
        nc.sync.dma_start(x_v[:, :, 0:512], folded(xT, slice(0, 512)))
        nc.scalar.dma_start(wq_v[:, :, 0:128], folded(wqT, slice(0, 128)))
        nc.gpsimd.dma_start(x_v[:, :, 1024:2048],
                            folded(xT, slice(1024, 2048)))
        nc.sync.dma_start(wk_v[:, :, 0:128], folded(wkT, slice(0, 128)))
        nc.scalar.dma_start(wv_b[:].rearrange("p (kc j) -> p kc j", kc=4),
                            folded(wvT, slice(0, DIM)))
        nc.sync.dma_start(wk_v[:, :, 128:512], folded(wkT, slice(128, DIM)))
        nc.scalar.dma_start(x_v[:, :, 512:1024], folded(xT, slice(512, 1024)))
        nc.sync.dma_start(wq_v[:, :, 128:512], folded(wqT, slice(128, DIM)))
        em_dma(nc.scalar, 0, 0)
        em_dma(nc.sync, 1, 0)
        em_dma(nc.scalar, 2, 0)
        em_dma(nc.sync, 3, 0)
        nc.gpsimd.dma_start(pj_b[:].rearrange("p (kc j) -> p kc j", kc=4),
                            folded(projT, slice(0, DIM)))
        nc.scalar.dma_start(bias_sb[:], biasb[:])
        for g in range(4):
            em_dma((nc.sync, nc.scalar)[g % 2], g, 1)


        # ---- projection emitters (single [128,512] PSUM stage) ----
        def emit_q_half(m, c):
            ms = slice(m * 128, (m + 1) * 128)
            cs = slice(c * 512, (c + 1) * 512)
            ps = ps_proj.tile([128, 512], F32, name=f"psq{m}_{c}", tag="proj")
            for kc in range(4):
                nc.tensor.matmul(ps[:], wq_b[:, kc * DIM:][:, ms],
                                 x_b[:, kc * N:][:, cs],
                                 start=(kc == 0), stop=(kc == 3))
            nc.vector.tensor_copy(q_sb[m][:, cs], ps[:])

        def emit_k_half(m, khalf, c2):
            ms = slice(m * 128, (m + 1) * 128)
            co = khalf * 1024 + c2 * 512
            ps = ps_proj.tile([128, 512], F32, name=f"psk{m}_{khalf}_{c2}",
                              tag="proj")
            for kc in range(4):
                nc.tensor.matmul(ps[:], wk_b[:, kc * DIM:][:, ms],
                                 x_b[:, kc * N + co:][:, 0:512],
                                 start=(kc == 0), stop=(kc == 3))
            nc.vector.tensor_copy(k_sb[m][:, co:co + 512], ps[:])

        def emit_v_proj(kt):
            ks = slice(kt * 128, (kt + 1) * 128)
            ps = ps_proj.tile([128, 512], F32, name=f"psv{kt}", tag="proj")
            for kc in range(4):
                nc.tensor.matmul(ps[:], x_b[:, kc * N:][:, ks],
                                 wv_b[:, kc * DIM:(kc + 1) * DIM],
                                 start=(kc == 0), stop=(kc == 3))
            nc.vector.tensor_copy(
                v_sb[kt][:].rearrange("p (h c) -> p h c", h=8)[:, :, 0:64],
                ps[:].rearrange("p (h c) -> p h c", h=8))

        def q(m, c):
            return lambda: emit_q_half(m, c)

        def k(m, h, c):
            return lambda: emit_k_half(m, h, c)

        def v(kt):
            return lambda: emit_v_proj(kt)

        # weave: (it, ktp) -> independent PE filler. Emit deadlines:
        # k(p,kh,c2) before scores it=p(+4) ktp=2*kh+c2; v(kt) before
        # that kt's p@v (lag-2 -> ktp kt//2+3); q(m,1) before it4.
        weave = {
            (0, 0): [k(0, 0, 0), v(1), v(2)],
            (0, 1): [k(0, 0, 1), v(3), v(4)],
            (0, 2): [v(5), v(6), k(0, 1, 0)],
            (0, 3): [v(7), v(8)],
            (0, 4): [v(9), v(10), k(0, 1, 1)],
            (0, 5): [v(11), v(12), q(1, 0)],
            (0, 6): [v(13), v(14), k(1, 0, 0)],
            (0, 7): [v(15), k(1, 0, 1)],
            (1, 0): [k(1, 1, 0)],
            (1, 1): [k(1, 1, 1)],
            (1, 2): [q(2, 0)],
            (1, 3): [k(2, 0, 0)],
            (1, 4): [k(2, 0, 1)],
            (1, 5): [k(2, 1, 0)],
            (1, 6): [k(2, 1, 1)],
            (1, 7): [q(3, 0)],
            (2, 0): [k(3, 0, 0)],
            (2, 1): [k(3, 0, 1)],
            (2, 2): [k(3, 1, 0)],
            (2, 3): [k(3, 1, 1)],
            (2, 4): [q(0, 1)],
            (2, 5): [q(1, 1)],
            (2, 6): [q(2, 1)],
            (2, 7): [q(3, 1)],
        }

        # ---- prologue: bare minimum for the first scores+exp.
        #      k for kt0 only (4x128-col matmuls) so the first score
        #      fires ~3us earlier; the full k(0,0,0) follows in-loop ----
        emit_q_half(0, 0)
        ps0 = ps_proj.tile([128, 512], F32, name="psk_first", tag="proj")
        for kc in range(4):
            nc.tensor.matmul(ps0[:, 0:256], wk_b[:, kc * DIM:][:, 0:128],
                             x_b[:, kc * N:][:, 0:256],
                             start=(kc == 0), stop=(kc == 3))
        nc.vector.tensor_copy(k_sb[0][:, 0:256], ps0[:, 0:256])
        emit_v_proj(0)

        # ---- norm + phase3 emitters ----
        def emit_norm_a(pair, hq, pv):
            # sums rows -> SBUF, partition-broadcast via two K=1 matmuls
            srow = small.tile([1, 1024], BF16,
                              name=f"sr{pair}_{hq.start}", tag="sr")
            for hi in range(2):
                nc.vector.tensor_copy(
                    srow[0:1, hi * 512:(hi + 1) * 512], pv[hi][64:65, :])
            bc_ps = ps_proj.tile([128, 512], F32,
                                 name=f"bc{pair}_{hq.start}", tag="proj")
            for hi in range(2):
                nc.tensor.matmul(
                    bc_ps[hi * 64:(hi + 1) * 64, :], ones_row[:],
                    srow[0:1, hi * 512:(hi + 1) * 512],
                    start=True, stop=True, tile_position=(0, hi * 64))
            return bc_ps

        def emit_norm_b(pair, hq, pv, bc_ps):
            rc = small.tile([128, 512], F32,
                            name=f"rc{pair}_{hq.start}", tag="rc")
            nc.vector.reciprocal_approx_fast(rc[:], bc_ps[:])
            for hi in range(2):
                nc.vector.tensor_mul(
                    uhat[pair][hi * 64:(hi + 1) * 64, hq],
                    pv[hi][0:64, :], rc[hi * 64:(hi + 1) * 64, :])

        def emit_phase3(m):
            ms = slice(m * 128, (m + 1) * 128)
            pp = ps_proj.tile([128, 512], F32, name=f"pp{m}", tag="proj")
            for kc in range(4):
                nc.tensor.matmul(pp[:], uhat[kc][:, ms],
                                 pj_b[:, kc * DIM:(kc + 1) * DIM],
                                 start=(kc == 0), stop=(kc == 3))
            ob = osb.tile([128, DIM], F32, name=f"ob{m}", tag="ob")
            nc.vector.tensor_add(ob[:], pp[:], bias_sb[:])
            (nc.sync, nc.gpsimd)[m % 2].dma_start(out[ms, :], ob[:])

        # ---- main loop: 8 iterations of (pair, half) x 8 kt-pairs ----
        # Per ktp j: scores(2j), exp(2j), scores(2j+1), exp(2j+1),
        # mul(j) [gpsimd for j==2], then scheduled PV pairs (lag 2-3)
        # and deferred norm / phase3 work.
        prev = [None]       # (pair, hq, pv, pending pv-emitters [6],[7])
        bc_hold = [None]

        for it in range(8):
            half, pair = it // 4, it % 4
            hq = slice(half * 512, (half + 1) * 512)
            pv = [ps_pv.tile([128, 512], F32,
                             name=f"pv{pair}_{half}_{hi}", tag="pv")
                  for hi in range(2)]
            pv_emit = {}      # j -> emitter closure
            if it == 0:
                own_sched = {5: [0], 6: [1], 7: [2]}
                carry_sched, norm_ktp, first_j = {}, None, 0
            elif it == 1:
                own_sched = {4: [1], 5: [0], 6: [2, 3], 7: [4]}
                carry_sched = {0: [3], 1: [4], 2: [5, 6], 3: [7]}
                norm_ktp, first_j = 3, 1
            elif it == 2:
                own_sched = {3: [1], 4: [2], 5: [0, 3], 6: [4], 7: [5]}
                carry_sched = {0: [5], 1: [6], 2: [7]}
                norm_ktp, first_j = 2, 1
            else:
                own_sched = {3: [1], 4: [2], 5: [0, 3], 6: [4], 7: [5]}
                carry_sched = {0: [6], 1: [7]}
                norm_ktp, first_j = 1, 1

            for ktp in range(NKTP):
                # -- scores + exp for kt = 2*ktp, 2*ktp+1; one mul --
                praw = praw_p.tile([128, 2048], BF16,
                                   name=f"pr{it}_{ktp}", tag="pr")
                for t in range(2):
                    kt = 2 * ktp + t
                    kts = slice(kt * 128, (kt + 1) * 128)
                    st = ps_score.tile([128, 1024], F32,
                                       name=f"st{it}_{kt}", tag="score")
                    for hi in range(2):
                        po = hi * 64
                        pos = slice(po, po + 64)
                        nc.tensor.matmul(
                            st[:, hi * 512:(hi + 1) * 512],
                            k_sb[pair][pos, kts], q_sb[pair][pos, hq],
                            start=True, stop=True, tile_position=(po, 0))
                    nc.scalar.activation(
                        praw[:, t * 1024:(t + 1) * 1024], st[:], AF.Exp)
                phat = phat_p.tile([128, 2048], BF16,
                                   name=f"ph{it}_{ktp}", tag="ph")
                g, i0 = ktp // 2, (ktp % 2) * 2
                em2 = em_g[g][half][:].rearrange("p (i q) -> p i q", i=4)[
                    :, i0:i0 + 2, :].unsqueeze(2).broadcast_to(
                    [128, 2, 2, 512])
                # ktp0's mul runs on GpSimd (SBUF-only op, ~4us): issued
                # early enough that its PV (scheduled at ktp5) is ready
                eng = nc.gpsimd if ktp == 0 else nc.vector
                eng.tensor_mul(
                    phat[:].rearrange("p (t h q) -> p t h q", t=2, h=2),
                    praw[:].rearrange("p (t h q) -> p t h q", t=2, h=2), em2)

                def mk_pv(j, phat=phat, pv=pv, pair=pair):
                    def emit(first, last):
                        # start/stop follow ISSUE order (accumulation
                        # groups must begin with the start matmul)
                        for t in range(2):
                            kt = 2 * j + t
                            for hi in range(2):
                                h = 2 * pair + hi
                                nc.tensor.matmul(
                                    pv[hi][0:65, :],
                                    v_sb[kt][:, h * 65:(h + 1) * 65],
                                    phat[:, (2 * t + hi) * 512:][:, 0:512],
                                    start=(first and t == 0),
                                    stop=(last and t == 1))
                    return emit
                pv_emit[ktp] = mk_pv(ktp)

                # -- deferred work: iteration-dependent schedule. it0's
                #    PV backlog cascades into it1/it2 (lag up to 5 ktps,
                #    phat ring = 6) to unload it0's emit-heavy PE. --
                if prev[0] is not None:
                    for j in carry_sched.get(ktp, ()):
                        prev[0][3][j](False, j == 7)     # prev-it PVs
                    if ktp == norm_ktp:
                        p_pair, p_hq, p_pv, _ = prev[0]
                        bc_hold[0] = emit_norm_a(p_pair, p_hq, p_pv)
                    if ktp == norm_ktp + 1:
                        p_pair, p_hq, p_pv, _ = prev[0]
                        emit_norm_b(p_pair, p_hq, p_pv, bc_hold[0])
                        prev[0] = None
                        bc_hold[0] = None
                for fn in weave.get((it, ktp), ()):
                    fn()
                if half == 1 and ktp == 4:
                    emit_phase3(pair)                    # half-0 block
                # -- lagged p@v issue (DVE mul(j) ready ~ktp j+1.9;
                #    pool mul(0) ~ktp 5) --
                for j in own_sched.get(ktp, ()):
                    pv_emit[j](j == first_j, False)

            prev[0] = (pair, hq, pv, pv_emit)

        # ---- tail: last PV flush, norm, half-1 phase3 blocks (these
        #      read uhat[kc][:, 512:...] for ALL kc, so they need every
        #      half-1 norm including the last iteration's) ----
        p_pair, p_hq, p_pv, p_em = prev[0]
        p_em[6](False, False)
        p_em[7](False, True)
        bc = emit_norm_a(p_pair, p_hq, p_pv)
        emit_norm_b(p_pair, p_hq, p_pv, bc)
        for m in range(4, 8):
            emit_phase3(m)

    nc.compile()
    return nc


@functools.lru_cache(maxsize=1)
def _get_nc():
    return build()


def _prep_inputs(x, attn_mask, qkv_w, proj_w, proj_b):
    x = np.asarray(x, dtype=np.float32)
    mask = np.asarray(attn_mask, dtype=np.float32).reshape(N, N)
    qkv_w = np.asarray(qkv_w, dtype=np.float32)
    proj_w = np.asarray(proj_w, dtype=np.float32)
    proj_b = np.asarray(proj_b, dtype=np.float32)

    bf = ml_dtypes.bfloat16
    wqT = np.ascontiguousarray((qkv_w[0:DIM] * SCALE).T).astype(bf)
    wkT = np.ascontiguousarray(qkv_w[DIM:2 * DIM].T).astype(bf)
    wvT = np.ascontiguousarray(qkv_w[2 * DIM:3 * DIM].T).astype(bf)
    projT = np.ascontiguousarray(proj_w.T).astype(bf)
    biasb = np.tile(proj_b, (128, 1))

    expm = np.exp(mask)
    # per-core key permutation: own seq-half first, other half second, so
    # the query chunk is always columns [0, NQ) of the permuted x.T
    xTs = {}
    emTs = {}
    for s in range(2):
        o = 1 - s
        emT = np.ascontiguousarray(expm[s * NQ:(s + 1) * NQ, :].T)  # [keys, q]
        emTs[s] = np.concatenate(
            [emT[s * NQ:(s + 1) * NQ], emT[o * NQ:(o + 1) * NQ]], axis=0
        ).astype(bf)
        for b in range(B):
            xTb = x[b].T  # [DIM, N]
            xTs[(b, s)] = np.ascontiguousarray(np.concatenate(
                [xTb[:, s * NQ:(s + 1) * NQ], xTb[:, o * NQ:(o + 1) * NQ]],
                axis=1)).astype(bf)

    in_maps = []
    for c in range(NCORES):
        b, s = c // 2, c % 2
        in_maps.append({
            "xT": xTs[(b, s)],
            "wqT": wqT, "wkT": wkT, "wvT": wvT, "projT": projT,
            "biasb": biasb, "expmT": emTs[s],
        })
    return in_maps


def run(inputs, trace=False, tmpdir=None):
    nc = _get_nc()
    in_maps = _prep_inputs(**inputs)
    res = run_bass_kernel_spmd(nc, in_maps, core_ids=list(range(NCORES)),
                               trace=trace, tmpdir=tmpdir)
    full = np.empty((B, N, DIM), dtype=np.float32)
    for c in range(NCORES):
        b, s = c // 2, c % 2
        full[b, s * NQ:(s + 1) * NQ, :] = res.results[c]["out"]
    return full, res


def kernel(**inputs) -> np.ndarray:
    return run(inputs)[0]


# revision 22
# speedup vs baseline: 1.0031x; 1.0031x over previous
"""Trainium2 Bass kernel for nn_Attention (B=4, N=2048, DIM=512, H=8).

Sharding: 8 cores = (batch b, seq-half s). Each core computes attention
outputs for queries [s*1024, (s+1)*1024) of batch b, all 8 heads, plus
the output projection for those rows. Outputs are disjoint -> host
gather is a pure concatenation (no reduction). Keys are permuted per
core (own seq-half first) so the query chunk is always columns [0, NQ)
of the permuted x.T; attention is permutation-invariant over keys.

Dataflow (matmul operands bf16, PSUM accumulation f32): the Scalar
engine's 128 exp tiles (~1.1us each) are the hard floor; everything is
scheduled around keeping the exp pipeline dense:

  - inputs land via 13 batched DMA triggers (priority-ordered; the
    trigger instruction itself costs ~620ns of engine time, so fewer +
    earlier triggers move the first exp from ~30us to ~8us)
  - PSUM: scores ring 2x[128,1024] (4 banks) fed only by score matmuls
    and drained only by exp; pv accumulators ring 2x[128,512] (2
    banks, both heads of a pair share the ring across iterations); a
    2x[128,512] staging ring (2 banks) for projection emits / the
    norm broadcast / phase3
  - per kt-pair: 2 score tiles -> 2 exps -> ONE [128,2048] mask-mul
    (praw * exp(mask), both kts at once, 2x DVE mode); one mul per
    iteration runs on GpSimd (SBUF-only op) to unload the DVE
  - p@v is issued with a 2-ktp lag so its phat operand is always
    ready (the PE queue is in-order; a blocked head-of-line matmul
    would stall independent work queued behind it)
  - QKV/proj projections are woven between score matmuls as
    independent PE filler; phase3 (out proj) is interleaved into the
    half-1 iterations so the tail after the last exp is ~1 block
  - normalization: softmax sums ride in PSUM row 64 (ones column in
    v), partition-broadcast via two K=1 PE matmuls, reciprocal + mul
    on DVE, deferred into the next iteration's first ktp slots
"""
import functools
import numpy as np
import ml_dtypes
from contextlib import ExitStack

import concourse.bass as bass
import concourse.tile as tile
from concourse import bacc, mybir
from concourse.bass_utils import run_bass_kernel_spmd

F32 = mybir.dt.float32
BF16 = mybir.dt.bfloat16
AF = mybir.ActivationFunctionType

B, N, DIM, H, D = 4, 2048, 512, 8, 64
SCALE = D ** -0.5
NQ = N // 2          # queries per core
NKT = N // 128       # key tiles (16)
NKTP = NKT // 2      # kt pairs per iteration (8)
NCORES = 8


def build(dbg=False):
    nc = bacc.Bacc("TRN2", target_bir_lowering=False, debug=False,
                   num_devices=NCORES)
    xT = nc.dram_tensor("xT", [DIM, N], BF16, kind="ExternalInput").ap()
    wqT = nc.dram_tensor("wqT", [DIM, DIM], BF16, kind="ExternalInput").ap()
    wkT = nc.dram_tensor("wkT", [DIM, DIM], BF16, kind="ExternalInput").ap()
    wvT = nc.dram_tensor("wvT", [DIM, DIM], BF16, kind="ExternalInput").ap()
    projT = nc.dram_tensor("projT", [DIM, DIM], BF16, kind="ExternalInput").ap()
    biasb = nc.dram_tensor("biasb", [128, DIM], F32, kind="ExternalInput").ap()
    expmT = nc.dram_tensor("expmT", [N, NQ], BF16, kind="ExternalInput").ap()
    out = nc.dram_tensor("out", [NQ, DIM], F32, kind="ExternalOutput").ap()

    with tile.TileContext(nc) as tc, ExitStack() as ctx:
        # ---- SBUF pools ----
        wp = ctx.enter_context(tc.tile_pool(name="wp", bufs=1))
        kv = ctx.enter_context(tc.tile_pool(name="kv", bufs=1))
        small = ctx.enter_context(tc.tile_pool(name="small", bufs=2))
        osb = ctx.enter_context(tc.tile_pool(name="osb", bufs=2))
        praw_p = ctx.enter_context(tc.tile_pool(name="praw", bufs=6))
        phat_p = ctx.enter_context(tc.tile_pool(name="phat", bufs=6))
        # ---- PSUM pools: 2x2 + 2x1 + 2x1 = 8 banks ----
        ps_score = ctx.enter_context(
            tc.tile_pool(name="ps_score", bufs=2, space="PSUM"))  # 2x2 banks
        ps_pv = ctx.enter_context(
            tc.tile_pool(name="ps_pv", bufs=2, space="PSUM"))     # 2x1 bank
        ps_proj = ctx.enter_context(
            tc.tile_pool(name="ps_proj", bufs=2, space="PSUM"))   # 2x1 bank

        # ---- persistent tiles (combined so one DMA fills each) ----
        wq_b = wp.tile([128, 4 * DIM], BF16, name="wq_b", tag="wq_b")
        wk_b = wp.tile([128, 4 * DIM], BF16, name="wk_b", tag="wk_b")
        wv_b = wp.tile([128, 4 * DIM], BF16, name="wv_b", tag="wv_b")
        pj_b = wp.tile([128, 4 * DIM], BF16, name="pj_b", tag="pj_b")
        x_b = wp.tile([128, 4 * N], BF16, name="x_b", tag="x_b")
        bias_sb = wp.tile([128, DIM], F32, name="bias_sb", tag="bias_sb")
        ones_row = wp.tile([1, 64], BF16, name="ones_row", tag="ones_row")
        warm = wp.tile([1, 2], F32, name="warm", tag="warm")
        # em groups: (g, qhalf) holds kt = 4g..4g+3 for one query half,
        # layout [128, (i, q512)] -- split by q-half so only 2MB of mask
        # is needed during the first four iterations
        em_g = [[wp.tile([128, 4 * 512], BF16, name=f"em{g}_{h}",
                         tag=f"em{g}_{h}") for h in range(2)]
                for g in range(4)]
        q_sb = [kv.tile([128, NQ], BF16, name=f"q{m}", tag=f"q{m}")
                for m in range(4)]
        k_sb = [kv.tile([128, N], BF16, name=f"k{m}", tag=f"k{m}")
                for m in range(4)]
        # per-head 65th column is ones -> sums row lands in PSUM row 64
        v_sb = [kv.tile([128, 8 * 65], BF16, name=f"v{kt}", tag=f"v{kt}")
                for kt in range(NKT)]
        uhat = [kv.tile([128, NQ], BF16, name=f"uh{p}", tag=f"uh{p}")
                for p in range(4)]

        # warm the activation table (implicit ACT_TABLE_LOAD happens on
        # the first Exp; issue it at t=0 so the ~1.3us load overlaps DMA)
        nc.gpsimd.memset(warm[:], 0.0)
        nc.scalar.activation(warm[:], warm[:], AF.Exp)

        nc.gpsimd.memset(ones_row[:], 1.0)
        for kt in range(NKT):
            nc.gpsimd.memset(
                v_sb[kt][:].rearrange("p (h c) -> p h c", h=8)[:, :, 64:65],
                1.0)

        # ---- input DMAs: batched (one trigger per tensor/chunk),
        #      priority-ordered by first use ----
        def folded(src, cols):
            return src[:, cols].rearrange("(kc p) j -> p kc j", kc=4)

        # Criticality order, interleaved across the three DGE queues so
        # the first-needed tensors drain in parallel:
        #   q(0,0) needs wq+x1a; k(0,0,0) needs wk; v(0) needs wv;
        #   em half-0 during it0-3; x2 by it0/ktp2; em half-1 from it4.
        x_v = x_b[:].rearrange("p (kc j) -> p kc j", kc=4)

        def em_dma(eng, g, h):
            src = expmT[g * 512:(g + 1) * 512, h * 512:(h + 1) * 512]
            eng.dma_start(
                em_g[g][h][:].rearrange("p (i q) -> p i q", i=4),
                src.rearrange("(i p) q -> p i q", i=4))

        wq_v = wq_b[:].rearrange("p (kc j) -> p kc j", kc=4)
        wk_v = wk_b[:].rearrange("p (kc j) -> p kc j", kc=4)
        # first m-block of wq/wk ships separately (128KB) so the
        # prologue emits unblock as early as possible
        nc.sync.dma_start(x_v[:, :, 0:512], folded(xT, slice(0, 512)))
        nc.scalar.dma_start(wq_v[:, :, 0:128], folded(wqT, slice(0, 128)))
        nc.gpsimd.dma_start(x_v[:, :, 1024:2048],
                            folded(xT, slice(1024, 2048)))
        nc.sync.dma_start(wk_v[:, :, 0:128], folded(wkT, slice(0, 128)))
        nc.scalar.dma_start(wv_b[:].rearrange("p (kc j) -> p kc j", kc=4),
                            folded(wvT, slice(0, DIM)))
        nc.sync.dma_start(wk_v[:, :, 128:512], folded(wkT, slice(128, DIM)))
        nc.scalar.dma_start(x_v[:, :, 512:1024], folded(xT, slice(512, 1024)))
        nc.sync.dma_start(wq_v[:, :, 128:512], folded(wqT, slice(128, DIM)))
        em_dma(nc.scalar, 0, 0)
        em_dma(nc.sync, 1, 0)
        em_dma(nc.scalar, 2, 0)
        em_dma(nc.sync, 3, 0)
        nc.gpsimd.dma_start(pj_b[:].rearrange("p (kc j) -> p kc j", kc=4),
                            folded(projT, slice(0, DIM)))
        nc.scalar.dma_start(bias_sb[:], biasb[:])
        for g in range(4):
            em_dma((nc.sync, nc.scalar)[g % 2], g, 1)


        # ---- projection emitters (single [128,512] PSUM stage) ----
        def emit_q_half(m, c):
            ms = slice(m * 128, (m + 1) * 128)
            cs = slice(c * 512, (c + 1) * 512)
            ps = ps_proj.tile([128, 512], F32, name=f"psq{m}_{c}", tag="proj")
            for kc in range(4):
                nc.tensor.matmul(ps[:], wq_b[:, kc * DIM:][:, ms],
                                 x_b[:, kc * N:][:, cs],
                                 start=(kc == 0), stop=(kc == 3))
            nc.vector.tensor_copy(q_sb[m][:, cs], ps[:])

        def emit_k_half(m, khalf, c2):
            ms = slice(m * 128, (m + 1) * 128)
            co = khalf * 1024 + c2 * 512
            ps = ps_proj.tile([128, 512], F32, name=f"psk{m}_{khalf}_{c2}",
                              tag="proj")
            for kc in range(4):
                nc.tensor.matmul(ps[:], wk_b[:, kc * DIM:][:, ms],
                                 x_b[:, kc * N + co:][:, 0:512],
                                 start=(kc == 0), stop=(kc == 3))
            nc.vector.tensor_copy(k_sb[m][:, co:co + 512], ps[:])

        def emit_v_proj(kt):
            ks = slice(kt * 128, (kt + 1) * 128)
            ps = ps_proj.tile([128, 512], F32, name=f"psv{kt}", tag="proj")
            for kc in range(4):
                nc.tensor.matmul(ps[:], x_b[:, kc * N:][:, ks],
                                 wv_b[:, kc * DIM:(kc + 1) * DIM],
                                 start=(kc == 0), stop=(kc == 3))
            nc.vector.tensor_copy(
                v_sb[kt][:].rearrange("p (h c) -> p h c", h=8)[:, :, 0:64],
                ps[:].rearrange("p (h c) -> p h c", h=8))

        def q(m, c):
            return lambda: emit_q_half(m, c)

        def k(m, h, c):
            return lambda: emit_k_half(m, h, c)

        def v(kt):
            return lambda: emit_v_proj(kt)

        # weave: (it, ktp) -> independent PE filler. Emit deadlines:
        # k(p,kh,c2) before scores it=p(+4) ktp=2*kh+c2; v(kt) before
        # that kt's p@v (lag-2 -> ktp kt//2+3); q(m,1) before it4.
        weave = {
            (0, 0): [k(0, 0, 0), v(1), v(2)],
            (0, 1): [k(0, 0, 1), v(3), v(4)],
            (0, 2): [v(5), v(6), k(0, 1, 0)],
            (0, 3): [v(7), v(8)],
            (0, 4): [v(9), v(10), k(0, 1, 1)],
            (0, 5): [v(11), v(12), q(1, 0)],
            (0, 6): [v(13), v(14), k(1, 0, 0)],
            (0, 7): [v(15), k(1, 0, 1)],
            (1, 0): [k(1, 1, 0)],
            (1, 1): [k(1, 1, 1)],
            (1, 2): [q(2, 0)],
            (1, 3): [k(2, 0, 0)],
            (1, 4): [k(2, 0, 1)],
            (1, 5): [k(2, 1, 0)],
            (1, 6): [k(2, 1, 1)],
            (1, 7): [q(3, 0)],
            (2, 0): [k(3, 0, 0)],
            (2, 1): [k(3, 0, 1)],
            (2, 2): [k(3, 1, 0)],
            (2, 3): [k(3, 1, 1)],
            (2, 4): [q(0, 1)],
            (2, 5): [q(1, 1)],
            (2, 6): [q(2, 1)],
            (2, 7): [q(3, 1)],
        }

        # ---- prologue: bare minimum for the first scores+exp.
        #      k for kt0 only (4x128-col matmuls) so the first score
        #      fires ~3us earlier; the full k(0,0,0) follows in-loop ----
        emit_q_half(0, 0)
        ps0 = ps_proj.tile([128, 512], F32, name="psk_first", tag="proj")
        for kc in range(4):
            nc.tensor.matmul(ps0[:, 0:256], wk_b[:, kc * DIM:][:, 0:128],
                             x_b[:, kc * N:][:, 0:256],
                             start=(kc == 0), stop=(kc == 3))
        nc.vector.tensor_copy(k_sb[0][:, 0:256], ps0[:, 0:256])
        emit_v_proj(0)

        # ---- norm + phase3 emitters ----
        def emit_norm_a(pair, hq, pv):
            # sums rows -> SBUF, partition-broadcast via two K=1 matmuls
            srow = small.tile([1, 1024], BF16,
                              name=f"sr{pair}_{hq.start}", tag="sr")
            for hi in range(2):
                nc.vector.tensor_copy(
                    srow[0:1, hi * 512:(hi + 1) * 512], pv[hi][64:65, :])
            bc_ps = ps_proj.tile([128, 512], F32,
                                 name=f"bc{pair}_{hq.start}", tag="proj")
            for hi in range(2):
                nc.tensor.matmul(
                    bc_ps[hi * 64:(hi + 1) * 64, :], ones_row[:],
                    srow[0:1, hi * 512:(hi + 1) * 512],
                    start=True, stop=True, tile_position=(0, hi * 64))
            return bc_ps

        def emit_norm_b(pair, hq, pv, bc_ps):
            rc = small.tile([128, 512], F32,
                            name=f"rc{pair}_{hq.start}", tag="rc")
            nc.vector.reciprocal_approx_fast(rc[:], bc_ps[:])
            for hi in range(2):
                nc.vector.tensor_mul(
                    uhat[pair][hi * 64:(hi + 1) * 64, hq],
                    pv[hi][0:64, :], rc[hi * 64:(hi + 1) * 64, :])

        def emit_phase3(m):
            ms = slice(m * 128, (m + 1) * 128)
            pp = ps_proj.tile([128, 512], F32, name=f"pp{m}", tag="proj")
            for kc in range(4):
                nc.tensor.matmul(pp[:], uhat[kc][:, ms],
                                 pj_b[:, kc * DIM:(kc + 1) * DIM],
                                 start=(kc == 0), stop=(kc == 3))
            ob = osb.tile([128, DIM], F32, name=f"ob{m}", tag="ob")
            nc.vector.tensor_add(ob[:], pp[:], bias_sb[:])
            (nc.sync, nc.gpsimd)[m % 2].dma_start(out[ms, :], ob[:])

        # ---- main loop: 8 iterations of (pair, half) x 8 kt-pairs ----
        # Per ktp j: scores(2j), exp(2j), scores(2j+1), exp(2j+1),
        # mul(j) [gpsimd for j==2], then scheduled PV pairs (lag 2-3)
        # and deferred norm / phase3 work.
        prev = [None]       # (pair, hq, pv, pending pv-emitters [6],[7])
        bc_hold = [None]

        for it in range(8):
            half, pair = it // 4, it % 4
            hq = slice(half * 512, (half + 1) * 512)
            pv = [ps_pv.tile([128, 512], F32,
                             name=f"pv{pair}_{half}_{hi}", tag="pv")
                  for hi in range(2)]
            pv_emit = {}      # j -> emitter closure
            if it == 0:
                own_sched = {5: [0], 6: [1], 7: [2]}
                carry_sched, norm_ktp, first_j = {}, None, 0
            elif it == 1:
                own_sched = {4: [1], 5: [0], 6: [2, 3], 7: [4]}
                carry_sched = {0: [3], 1: [4], 2: [5, 6], 3: [7]}
                norm_ktp, first_j = 3, 1
            elif it == 2:
                own_sched = {3: [1], 4: [2], 5: [0, 3], 6: [4], 7: [5]}
                carry_sched = {0: [5], 1: [6], 2: [7]}
                norm_ktp, first_j = 2, 1
            else:
                own_sched = {3: [1], 4: [2], 5: [0, 3], 6: [4], 7: [5]}
                carry_sched = {0: [6], 1: [7]}
                norm_ktp, first_j = 1, 1

            for ktp in range(NKTP):
                # -- scores + exp for kt = 2*ktp, 2*ktp+1; one mul --
                praw = praw_p.tile([128, 2048], BF16,
                                   name=f"pr{it}_{ktp}", tag="pr")
                for t in range(2):
                    kt = 2 * ktp + t
                    kts = slice(kt * 128, (kt + 1) * 128)
                    st = ps_score.tile([128, 1024], F32,
                                       name=f"st{it}_{kt}", tag="score")
                    for hi in range(2):
                        po = hi * 64
                        pos = slice(po, po + 64)
                        nc.tensor.matmul(
                            st[:, hi * 512:(hi + 1) * 512],
                            k_sb[pair][pos, kts], q_sb[pair][pos, hq],
                            start=True, stop=True, tile_position=(po, 0))
                    nc.scalar.activation(
                        praw[:, t * 1024:(t + 1) * 1024], st[:], AF.Exp)
                phat = phat_p.tile([128, 2048], BF16,
                                   name=f"ph{it}_{ktp}", tag="ph")
                g, i0 = ktp // 2, (ktp % 2) * 2
                em2 = em_g[g][half][:].rearrange("p (i q) -> p i q", i=4)[
                    :, i0:i0 + 2, :].unsqueeze(2).broadcast_to(
                    [128, 2, 2, 512])
                # ktp0's mul runs on GpSimd (SBUF-only op, ~4us): issued
                # early enough that its PV (scheduled at ktp5) is ready
                eng = nc.gpsimd if ktp == 0 else nc.vector
                eng.tensor_mul(
                    phat[:].rearrange("p (t h q) -> p t h q", t=2, h=2),
                    praw[:].rearrange("p (t h q) -> p t h q", t=2, h=2), em2)

                def mk_pv(j, phat=phat, pv=pv, pair=pair):
                    def emit(first, last):
                        # start/stop follow ISSUE order (accumulation
                        # groups must begin with the start matmul)
                        for t in range(2):
                            kt = 2 * j + t
                            for hi in range(2):
                                h = 2 * pair + hi
                                nc.tensor.matmul(
                                    pv[hi][0:65, :],
                                    v_sb[kt][:, h * 65:(h + 1) * 65],
                                    phat[:, (2 * t + hi) * 512:][:, 0:512],
                                    start=(first and t == 0),
                                    stop=(last and t == 1))
                    return emit
                pv_emit[ktp] = mk_pv(ktp)

                # -- deferred work: iteration-dependent schedule. it0's
                #    PV backlog cascades into it1/it2 (lag up to 5 ktps,
                #    phat ring = 6) to unload it0's emit-heavy PE. --
                if prev[0] is not None:
                    for j in carry_sched.get(ktp, ()):
                        prev[0][3][j](False, j == 7)     # prev-it PVs
                    if ktp == norm_ktp:
                        p_pair, p_hq, p_pv, _ = prev[0]
                        bc_hold[0] = emit_norm_a(p_pair, p_hq, p_pv)
                    if ktp == norm_ktp + 1:
                        p_pair, p_hq, p_pv, _ = prev[0]
                        emit_norm_b(p_pair, p_hq, p_pv, bc_hold[0])
                        prev[0] = None
                        bc_hold[0] = None
                for fn in weave.get((it, ktp), ()):
                    fn()
                if half == 1 and ktp == 4:
                    emit_phase3(pair)                    # half-0 block
                # -- lagged p@v issue (DVE mul(j) ready ~ktp j+1.9;
                #    pool mul(0) ~ktp 5) --
                for j in own_sched.get(ktp, ()):
                    pv_emit[j](j == first_j, False)

            prev[0] = (pair, hq, pv, pv_emit)

        # ---- tail: last PV flush, norm, half-1 phase3 blocks (these
        #      read uhat[kc][:, 512:...] for ALL kc, so they need every
        #      half-1 norm including the last iteration's) ----
        p_pair, p_hq, p_pv, p_em = prev[0]
        p_em[6](False, False)
        p_em[7](False, True)
        bc = emit_norm_a(p_pair, p_hq, p_pv)
        emit_norm_b(p_pair, p_hq, p_pv, bc)
        for m in range(4, 8):
            emit_phase3(m)

    nc.compile()
    return nc


@functools.lru_cache(maxsize=1)
def _get_nc():
    return build()


def _prep_inputs(x, attn_mask, qkv_w, proj_w, proj_b):
    x = np.asarray(x, dtype=np.float32)
    mask = np.asarray(attn_mask, dtype=np.float32).reshape(N, N)
    qkv_w = np.asarray(qkv_w, dtype=np.float32)
    proj_w = np.asarray(proj_w, dtype=np.float32)
    proj_b = np.asarray(proj_b, dtype=np.float32)

    bf = ml_dtypes.bfloat16
    wqT = np.ascontiguousarray((qkv_w[0:DIM] * SCALE).T).astype(bf)
    wkT = np.ascontiguousarray(qkv_w[DIM:2 * DIM].T).astype(bf)
    wvT = np.ascontiguousarray(qkv_w[2 * DIM:3 * DIM].T).astype(bf)
    projT = np.ascontiguousarray(proj_w.T).astype(bf)
    biasb = np.tile(proj_b, (128, 1))

    expm = np.exp(mask)
    # per-core key permutation: own seq-half first, other half second, so
    # the query chunk is always columns [0, NQ) of the permuted x.T
    xTs = {}
    emTs = {}
    for s in range(2):
        o = 1 - s
        emT = np.ascontiguousarray(expm[s * NQ:(s + 1) * NQ, :].T)  # [keys, q]
        emTs[s] = np.concatenate(
            [emT[s * NQ:(s + 1) * NQ], emT[o * NQ:(o + 1) * NQ]], axis=0
        ).astype(bf)
        for b in range(B):
            xTb = x[b].T  # [DIM, N]
            xTs[(b, s)] = np.ascontiguousarray(np.concatenate(
                [xTb[:, s * NQ:(s + 1) * NQ], xTb[:, o * NQ:(o + 1) * NQ]],
                axis=1)).astype(bf)

    in_maps = []
    for c in range(NCORES):
        b, s = c // 2, c % 2
        in_maps.append({
            "xT": xTs[(b, s)],
            "wqT": wqT, "wkT": wkT, "wvT": wvT, "projT": projT,
            "biasb": biasb, "expmT": emTs[s],
        })
    return in_maps


def run(inputs, trace=False, tmpdir=None):
    nc = _get_nc()
    in_maps = _prep_inputs(**inputs)
    res = run_bass_kernel_spmd(nc, in_maps, core_ids=list(range(NCORES)),
                               trace=trace, tmpdir=tmpdir)
    full = np.empty((B, N, DIM), dtype=np.float32)
    for c in range(NCORES):
        b, s = c // 2, c % 2
        full[b, s * NQ:(s + 1) * NQ, :] = res.results[c]["out"]
    return full, res


def kernel(**inputs) -> np.ndarray:
    return run(inputs)[0]


# revision 23
# speedup vs baseline: 1.0097x; 1.0066x over previous
"""Trainium2 Bass kernel for nn_Attention (B=4, N=2048, DIM=512, H=8).

Sharding: 8 cores = (batch b, seq-half s). Each core computes attention
outputs for queries [s*1024, (s+1)*1024) of batch b, all 8 heads, plus
the output projection for those rows. Outputs are disjoint -> host
gather is a pure concatenation (no reduction). Keys are permuted per
core (own seq-half first) so the query chunk is always columns [0, NQ)
of the permuted x.T; attention is permutation-invariant over keys.

Dataflow (matmul operands bf16, PSUM accumulation f32): the Scalar
engine's 128 exp tiles (~1.1us each) are the hard floor; everything is
scheduled around keeping the exp pipeline dense:

  - inputs land via 13 batched DMA triggers (priority-ordered; the
    trigger instruction itself costs ~620ns of engine time, so fewer +
    earlier triggers move the first exp from ~30us to ~8us)
  - PSUM: scores ring 2x[128,1024] (4 banks) fed only by score matmuls
    and drained only by exp; pv accumulators ring 2x[128,512] (2
    banks, both heads of a pair share the ring across iterations); a
    2x[128,512] staging ring (2 banks) for projection emits / the
    norm broadcast / phase3
  - per kt-pair: 2 score tiles -> 2 exps -> ONE [128,2048] mask-mul
    (praw * exp(mask), both kts at once, 2x DVE mode); one mul per
    iteration runs on GpSimd (SBUF-only op) to unload the DVE
  - p@v is issued with a 2-ktp lag so its phat operand is always
    ready (the PE queue is in-order; a blocked head-of-line matmul
    would stall independent work queued behind it)
  - QKV/proj projections are woven between score matmuls as
    independent PE filler; phase3 (out proj) is interleaved into the
    half-1 iterations so the tail after the last exp is ~1 block
  - normalization: softmax sums ride in PSUM row 64 (ones column in
    v), partition-broadcast via two K=1 PE matmuls, reciprocal + mul
    on DVE, deferred into the next iteration's first ktp slots
"""
import functools
import numpy as np
import ml_dtypes
from contextlib import ExitStack

import concourse.bass as bass
import concourse.tile as tile
from concourse import bacc, mybir
from concourse.bass_utils import run_bass_kernel_spmd

F32 = mybir.dt.float32
BF16 = mybir.dt.bfloat16
AF = mybir.ActivationFunctionType

B, N, DIM, H, D = 4, 2048, 512, 8, 64
SCALE = D ** -0.5
NQ = N // 2          # queries per core
NKT = N // 128       # key tiles (16)
NKTP = NKT // 2      # kt pairs per iteration (8)
NCORES = 8


def build(dbg=False):
    nc = bacc.Bacc("TRN2", target_bir_lowering=False, debug=False,
                   num_devices=NCORES)
    xT = nc.dram_tensor("xT", [DIM, N], BF16, kind="ExternalInput").ap()
    wqT = nc.dram_tensor("wqT", [DIM, DIM], BF16, kind="ExternalInput").ap()
    wkT = nc.dram_tensor("wkT", [DIM, DIM], BF16, kind="ExternalInput").ap()
    wvT = nc.dram_tensor("wvT", [DIM, DIM], BF16, kind="ExternalInput").ap()
    projT = nc.dram_tensor("projT", [DIM, DIM], BF16, kind="ExternalInput").ap()
    biasb = nc.dram_tensor("biasb", [128, DIM], F32, kind="ExternalInput").ap()
    expmT = nc.dram_tensor("expmT", [N, NQ], BF16, kind="ExternalInput").ap()
    out = nc.dram_tensor("out", [NQ, DIM], F32, kind="ExternalOutput").ap()

    with tile.TileContext(nc) as tc, ExitStack() as ctx:
        # ---- SBUF pools ----
        wp = ctx.enter_context(tc.tile_pool(name="wp", bufs=1))
        kv = ctx.enter_context(tc.tile_pool(name="kv", bufs=1))
        small = ctx.enter_context(tc.tile_pool(name="small", bufs=2))
        osb = ctx.enter_context(tc.tile_pool(name="osb", bufs=2))
        praw_p = ctx.enter_context(tc.tile_pool(name="praw", bufs=6))
        phat_p = ctx.enter_context(tc.tile_pool(name="phat", bufs=6))
        # ---- PSUM pools: 2x2 + 2x1 + 2x1 = 8 banks ----
        ps_score = ctx.enter_context(
            tc.tile_pool(name="ps_score", bufs=2, space="PSUM"))  # 2x2 banks
        ps_pv = ctx.enter_context(
            tc.tile_pool(name="ps_pv", bufs=2, space="PSUM"))     # 2x1 bank
        ps_proj = ctx.enter_context(
            tc.tile_pool(name="ps_proj", bufs=2, space="PSUM"))   # 2x1 bank

        # ---- persistent tiles (combined so one DMA fills each) ----
        wq_b = wp.tile([128, 4 * DIM], BF16, name="wq_b", tag="wq_b")
        wk_b = wp.tile([128, 4 * DIM], BF16, name="wk_b", tag="wk_b")
        wv_b = wp.tile([128, 4 * DIM], BF16, name="wv_b", tag="wv_b")
        pj_b = wp.tile([128, 4 * DIM], BF16, name="pj_b", tag="pj_b")
        x_b = wp.tile([128, 4 * N], BF16, name="x_b", tag="x_b")
        bias_sb = wp.tile([128, DIM], F32, name="bias_sb", tag="bias_sb")
        ones_row = wp.tile([1, 64], BF16, name="ones_row", tag="ones_row")
        warm = wp.tile([1, 2], F32, name="warm", tag="warm")
        # em groups: (g, qhalf) holds kt = 4g..4g+3 for one query half,
        # layout [128, (i, q512)] -- split by q-half so only 2MB of mask
        # is needed during the first four iterations
        em_g = [[wp.tile([128, 4 * 512], BF16, name=f"em{g}_{h}",
                         tag=f"em{g}_{h}") for h in range(2)]
                for g in range(4)]
        q_sb = [kv.tile([128, NQ], BF16, name=f"q{m}", tag=f"q{m}")
                for m in range(4)]
        k_sb = [kv.tile([128, N], BF16, name=f"k{m}", tag=f"k{m}")
                for m in range(4)]
        # per-head 65th column is ones -> sums row lands in PSUM row 64
        v_sb = [kv.tile([128, 8 * 65], BF16, name=f"v{kt}", tag=f"v{kt}")
                for kt in range(NKT)]
        uhat = [kv.tile([128, NQ], BF16, name=f"uh{p}", tag=f"uh{p}")
                for p in range(4)]

        # warm the activation table (implicit ACT_TABLE_LOAD happens on
        # the first Exp; issue it at t=0 so the ~1.3us load overlaps DMA)
        nc.gpsimd.memset(warm[:], 0.0)
        nc.scalar.activation(warm[:], warm[:], AF.Exp)

        nc.gpsimd.memset(ones_row[:], 1.0)
        for kt in range(NKT):
            nc.gpsimd.memset(
                v_sb[kt][:].rearrange("p (h c) -> p h c", h=8)[:, :, 64:65],
                1.0)

        # ---- input DMAs: batched (one trigger per tensor/chunk),
        #      priority-ordered by first use ----
        def folded(src, cols):
            return src[:, cols].rearrange("(kc p) j -> p kc j", kc=4)

        # Criticality order, interleaved across the three DGE queues so
        # the first-needed tensors drain in parallel:
        #   q(0,0) needs wq+x1a; k(0,0,0) needs wk; v(0) needs wv;
        #   em half-0 during it0-3; x2 by it0/ktp2; em half-1 from it4.
        x_v = x_b[:].rearrange("p (kc j) -> p kc j", kc=4)

        def em_dma(eng, g, h):
            src = expmT[g * 512:(g + 1) * 512, h * 512:(h + 1) * 512]
            eng.dma_start(
                em_g[g][h][:].rearrange("p (i q) -> p i q", i=4),
                src.rearrange("(i p) q -> p i q", i=4))

        wq_v = wq_b[:].rearrange("p (kc j) -> p kc j", kc=4)
        wk_v = wk_b[:].rearrange("p (kc j) -> p kc j", kc=4)
        # first m-block of wq/wk ships separately (128KB) so the
        # prologue emits unblock as early as possible
        nc.sync.dma_start(x_v[:, :, 0:512], folded(xT, slice(0, 512)))
        nc.scalar.dma_start(wq_v[:, :, 0:128], folded(wqT, slice(0, 128)))
        nc.gpsimd.dma_start(x_v[:, :, 1024:2048],
                            folded(xT, slice(1024, 2048)))
        nc.sync.dma_start(wk_v[:, :, 0:128], folded(wkT, slice(0, 128)))
        nc.scalar.dma_start(wv_b[:].rearrange("p (kc j) -> p kc j", kc=4),
                            folded(wvT, slice(0, DIM)))
        nc.sync.dma_start(wk_v[:, :, 128:512], folded(wkT, slice(128, DIM)))
        nc.scalar.dma_start(x_v[:, :, 512:1024], folded(xT, slice(512, 1024)))
        nc.sync.dma_start(wq_v[:, :, 128:512], folded(wqT, slice(128, DIM)))
        em_dma(nc.scalar, 0, 0)
        em_dma(nc.sync, 1, 0)
        em_dma(nc.scalar, 2, 0)
        em_dma(nc.sync, 3, 0)
        nc.gpsimd.dma_start(pj_b[:].rearrange("p (kc j) -> p kc j", kc=4),
                            folded(projT, slice(0, DIM)))
        nc.scalar.dma_start(bias_sb[:], biasb[:])
        for g in range(4):
            em_dma((nc.sync, nc.scalar)[g % 2], g, 1)


        # ---- projection emitters (single [128,512] PSUM stage) ----
        def emit_q_half(m, c):
            ms = slice(m * 128, (m + 1) * 128)
            cs = slice(c * 512, (c + 1) * 512)
            ps = ps_proj.tile([128, 512], F32, name=f"psq{m}_{c}", tag="proj")
            for kc in range(4):
                nc.tensor.matmul(ps[:], wq_b[:, kc * DIM:][:, ms],
                                 x_b[:, kc * N:][:, cs],
                                 start=(kc == 0), stop=(kc == 3))
            nc.vector.tensor_copy(q_sb[m][:, cs], ps[:])

        def emit_k_half(m, khalf, c2):
            ms = slice(m * 128, (m + 1) * 128)
            co = khalf * 1024 + c2 * 512
            ps = ps_proj.tile([128, 512], F32, name=f"psk{m}_{khalf}_{c2}",
                              tag="proj")
            for kc in range(4):
                nc.tensor.matmul(ps[:], wk_b[:, kc * DIM:][:, ms],
                                 x_b[:, kc * N + co:][:, 0:512],
                                 start=(kc == 0), stop=(kc == 3))
            nc.vector.tensor_copy(k_sb[m][:, co:co + 512], ps[:])

        def emit_v_proj(kt):
            ks = slice(kt * 128, (kt + 1) * 128)
            ps = ps_proj.tile([128, 512], F32, name=f"psv{kt}", tag="proj")
            for kc in range(4):
                nc.tensor.matmul(ps[:], x_b[:, kc * N:][:, ks],
                                 wv_b[:, kc * DIM:(kc + 1) * DIM],
                                 start=(kc == 0), stop=(kc == 3))
            nc.vector.tensor_copy(
                v_sb[kt][:].rearrange("p (h c) -> p h c", h=8)[:, :, 0:64],
                ps[:].rearrange("p (h c) -> p h c", h=8))

        def q(m, c):
            return lambda: emit_q_half(m, c)

        def k(m, h, c):
            return lambda: emit_k_half(m, h, c)

        def v(kt):
            return lambda: emit_v_proj(kt)

        # weave: (it, ktp) -> independent PE filler. Emit deadlines:
        # k(p,kh,c2) before scores it=p(+4) ktp=2*kh+c2; v(kt) before
        # that kt's p@v (lag-2 -> ktp kt//2+3); q(m,1) before it4.
        weave = {
            (0, 0): [k(0, 0, 0), v(1), v(2)],
            (0, 1): [k(0, 0, 1), v(3), v(4)],
            (0, 2): [v(5), v(6), k(0, 1, 0)],
            (0, 3): [v(7), v(8)],
            (0, 4): [v(9), v(10), k(0, 1, 1)],
            (0, 5): [v(11), v(12), q(1, 0)],
            (0, 6): [v(13), v(14), k(1, 0, 0)],
            (0, 7): [v(15), k(1, 0, 1)],
            (1, 0): [k(1, 1, 0)],
            (1, 1): [k(1, 1, 1)],
            (1, 2): [q(2, 0)],
            (1, 3): [k(2, 0, 0)],
            (1, 4): [k(2, 0, 1)],
            (1, 5): [k(2, 1, 0)],
            (1, 6): [k(2, 1, 1)],
            (1, 7): [q(3, 0)],
            (2, 0): [k(3, 0, 0)],
            (2, 1): [k(3, 0, 1)],
            (2, 2): [k(3, 1, 0)],
            (2, 3): [k(3, 1, 1)],
            (2, 4): [q(0, 1)],
            (2, 5): [q(1, 1)],
            (2, 6): [q(2, 1)],
            (2, 7): [q(3, 1)],
        }

        # ---- prologue: bare minimum for the first scores+exp.
        #      k for kt0 only (4x128-col matmuls) so the first score
        #      fires ~3us earlier; the full k(0,0,0) follows in-loop ----
        emit_q_half(0, 0)
        ps0 = ps_proj.tile([128, 512], F32, name="psk_first", tag="proj")
        for kc in range(4):
            nc.tensor.matmul(ps0[:, 0:256], wk_b[:, kc * DIM:][:, 0:128],
                             x_b[:, kc * N:][:, 0:256],
                             start=(kc == 0), stop=(kc == 3))
        nc.vector.tensor_copy(k_sb[0][:, 0:256], ps0[:, 0:256])
        emit_v_proj(0)

        # ---- norm + phase3 emitters ----
        def emit_norm_a(pair, hq, pv):
            # sums rows -> SBUF, partition-broadcast via two K=1 matmuls
            srow = small.tile([1, 1024], BF16,
                              name=f"sr{pair}_{hq.start}", tag="sr")
            for hi in range(2):
                nc.vector.tensor_copy(
                    srow[0:1, hi * 512:(hi + 1) * 512], pv[hi][64:65, :])
            bc_ps = ps_proj.tile([128, 512], F32,
                                 name=f"bc{pair}_{hq.start}", tag="proj")
            for hi in range(2):
                nc.tensor.matmul(
                    bc_ps[hi * 64:(hi + 1) * 64, :], ones_row[:],
                    srow[0:1, hi * 512:(hi + 1) * 512],
                    start=True, stop=True, tile_position=(0, hi * 64))
            return bc_ps

        def emit_norm_b(pair, hq, pv, bc_ps):
            rc = small.tile([128, 512], F32,
                            name=f"rc{pair}_{hq.start}", tag="rc")
            nc.vector.reciprocal_approx_fast(rc[:], bc_ps[:])
            for hi in range(2):
                nc.vector.tensor_mul(
                    uhat[pair][hi * 64:(hi + 1) * 64, hq],
                    pv[hi][0:64, :], rc[hi * 64:(hi + 1) * 64, :])

        def emit_phase3(m):
            ms = slice(m * 128, (m + 1) * 128)
            pp = ps_proj.tile([128, 512], F32, name=f"pp{m}", tag="proj")
            for kc in range(4):
                nc.tensor.matmul(pp[:], uhat[kc][:, ms],
                                 pj_b[:, kc * DIM:(kc + 1) * DIM],
                                 start=(kc == 0), stop=(kc == 3))
            ob = osb.tile([128, DIM], F32, name=f"ob{m}", tag="ob")
            nc.vector.tensor_add(ob[:], pp[:], bias_sb[:])
            (nc.sync, nc.gpsimd)[m % 2].dma_start(out[ms, :], ob[:])

        # ---- main loop: 8 iterations of (pair, half) x 8 kt-pairs ----
        # Per ktp j: scores(2j), exp(2j), scores(2j+1), exp(2j+1),
        # mul(j) [gpsimd for j==2], then scheduled PV pairs (lag 2-3)
        # and deferred norm / phase3 work.
        prev = [None]       # (pair, hq, pv, pending pv-emitters [6],[7])
        bc_hold = [None]

        for it in range(8):
            half, pair = it // 4, it % 4
            hq = slice(half * 512, (half + 1) * 512)
            pv = [ps_pv.tile([128, 512], F32,
                             name=f"pv{pair}_{half}_{hi}", tag="pv")
                  for hi in range(2)]
            pv_emit = {}      # j -> emitter closure
            if it == 0:
                own_sched = {5: [0], 6: [1], 7: [2]}
                carry_sched, norm_ktp, first_j = {}, None, 0
            elif it == 1:
                own_sched = {4: [1], 5: [0], 6: [2, 3], 7: [4]}
                carry_sched = {0: [3], 1: [4], 2: [5, 6], 3: [7]}
                norm_ktp, first_j = 3, 1
            elif it == 2:
                own_sched = {3: [1], 4: [2], 5: [0, 3], 6: [4], 7: [5]}
                carry_sched = {0: [5], 1: [6], 2: [7]}
                norm_ktp, first_j = 2, 1
            else:
                own_sched = {3: [1], 4: [2], 5: [0, 3], 6: [4], 7: [5]}
                carry_sched = {0: [6], 1: [7]}
                norm_ktp, first_j = 1, 1

            for ktp in range(NKTP):
                # -- scores + exp for kt = 2*ktp, 2*ktp+1; one mul --
                praw = praw_p.tile([128, 2048], BF16,
                                   name=f"pr{it}_{ktp}", tag="pr")
                for t in range(2):
                    kt = 2 * ktp + t
                    kts = slice(kt * 128, (kt + 1) * 128)
                    st = ps_score.tile([128, 1024], F32,
                                       name=f"st{it}_{kt}", tag="score")
                    for hi in range(2):
                        po = hi * 64
                        pos = slice(po, po + 64)
                        nc.tensor.matmul(
                            st[:, hi * 512:(hi + 1) * 512],
                            k_sb[pair][pos, kts], q_sb[pair][pos, hq],
                            start=True, stop=True, tile_position=(po, 0))
                    nc.scalar.activation(
                        praw[:, t * 1024:(t + 1) * 1024], st[:], AF.Exp)
                phat = phat_p.tile([128, 2048], BF16,
                                   name=f"ph{it}_{ktp}", tag="ph")
                g, i0 = ktp // 2, (ktp % 2) * 2
                em2 = em_g[g][half][:].rearrange("p (i q) -> p i q", i=4)[
                    :, i0:i0 + 2, :].unsqueeze(2).broadcast_to(
                    [128, 2, 2, 512])
                # ktp0's mul runs on GpSimd (SBUF-only op, ~4us): issued
                # early enough that its PV (scheduled at ktp5) is ready
                eng = nc.gpsimd if ktp == 0 else nc.vector
                eng.tensor_mul(
                    phat[:].rearrange("p (t h q) -> p t h q", t=2, h=2),
                    praw[:].rearrange("p (t h q) -> p t h q", t=2, h=2), em2)

                def mk_pv(j, phat=phat, pv=pv, pair=pair):
                    def emit(first, last):
                        # start/stop follow ISSUE order (accumulation
                        # groups must begin with the start matmul)
                        for t in range(2):
                            kt = 2 * j + t
                            for hi in range(2):
                                h = 2 * pair + hi
                                nc.tensor.matmul(
                                    pv[hi][0:65, :],
                                    v_sb[kt][:, h * 65:(h + 1) * 65],
                                    phat[:, (2 * t + hi) * 512:][:, 0:512],
                                    start=(first and t == 0),
                                    stop=(last and t == 1))
                    return emit
                pv_emit[ktp] = mk_pv(ktp)

                # -- deferred work: iteration-dependent schedule. it0's
                #    PV backlog cascades into it1/it2 (lag up to 5 ktps,
                #    phat ring = 6) to unload it0's emit-heavy PE. --
                if prev[0] is not None:
                    for j in carry_sched.get(ktp, ()):
                        prev[0][3][j](False, j == 7)     # prev-it PVs
                    if ktp == norm_ktp:
                        p_pair, p_hq, p_pv, _ = prev[0]
                        bc_hold[0] = emit_norm_a(p_pair, p_hq, p_pv)
                    if ktp == norm_ktp + 1:
                        p_pair, p_hq, p_pv, _ = prev[0]
                        emit_norm_b(p_pair, p_hq, p_pv, bc_hold[0])
                        prev[0] = None
                        bc_hold[0] = None
                for fn in weave.get((it, ktp), ()):
                    fn()
                if half == 1 and ktp == 2:
                    emit_phase3(pair)                    # half-0 block
                # -- lagged p@v issue (DVE mul(j) ready ~ktp j+1.9;
                #    pool mul(0) ~ktp 5) --
                for j in own_sched.get(ktp, ()):
                    pv_emit[j](j == first_j, False)

            prev[0] = (pair, hq, pv, pv_emit)

        # ---- tail: last PV flush, norm, half-1 phase3 blocks (these
        #      read uhat[kc][:, 512:...] for ALL kc, so they need every
        #      half-1 norm including the last iteration's) ----
        p_pair, p_hq, p_pv, p_em = prev[0]
        p_em[6](False, False)
        p_em[7](False, True)
        bc = emit_norm_a(p_pair, p_hq, p_pv)
        emit_norm_b(p_pair, p_hq, p_pv, bc)
        for m in range(4, 8):
            emit_phase3(m)

    nc.compile()
    return nc


@functools.lru_cache(maxsize=1)
def _get_nc():
    return build()


def _prep_inputs(x, attn_mask, qkv_w, proj_w, proj_b):
    x = np.asarray(x, dtype=np.float32)
    mask = np.asarray(attn_mask, dtype=np.float32).reshape(N, N)
    qkv_w = np.asarray(qkv_w, dtype=np.float32)
    proj_w = np.asarray(proj_w, dtype=np.float32)
    proj_b = np.asarray(proj_b, dtype=np.float32)

    bf = ml_dtypes.bfloat16
    wqT = np.ascontiguousarray((qkv_w[0:DIM] * SCALE).T).astype(bf)
    wkT = np.ascontiguousarray(qkv_w[DIM:2 * DIM].T).astype(bf)
    wvT = np.ascontiguousarray(qkv_w[2 * DIM:3 * DIM].T).astype(bf)
    projT = np.ascontiguousarray(proj_w.T).astype(bf)
    biasb = np.tile(proj_b, (128, 1))

    expm = np.exp(mask)
    # per-core key permutation: own seq-half first, other half second, so
    # the query chunk is always columns [0, NQ) of the permuted x.T
    xTs = {}
    emTs = {}
    for s in range(2):
        o = 1 - s
        emT = np.ascontiguousarray(expm[s * NQ:(s + 1) * NQ, :].T)  # [keys, q]
        emTs[s] = np.concatenate(
            [emT[s * NQ:(s + 1) * NQ], emT[o * NQ:(o + 1) * NQ]], axis=0
        ).astype(bf)
        for b in range(B):
            xTb = x[b].T  # [DIM, N]
            xTs[(b, s)] = np.ascontiguousarray(np.concatenate(
                [xTb[:, s * NQ:(s + 1) * NQ], xTb[:, o * NQ:(o + 1) * NQ]],
                axis=1)).astype(bf)

    in_maps = []
    for c in range(NCORES):
        b, s = c // 2, c % 2
        in_maps.append({
            "xT": xTs[(b, s)],
            "wqT": wqT, "wkT": wkT, "wvT": wvT, "projT": projT,
            "biasb": biasb, "expmT": emTs[s],
        })
    return in_maps


def run(inputs, trace=False, tmpdir=None):
    nc = _get_nc()
    in_maps = _prep_inputs(**inputs)
    res = run_bass_kernel_spmd(nc, in_maps, core_ids=list(range(NCORES)),
                               trace=trace, tmpdir=tmpdir)
    full = np.empty((B, N, DIM), dtype=np.float32)
    for c in range(NCORES):
        b, s = c // 2, c % 2
        full[b, s * NQ:(s + 1) * NQ, :] = res.results[c]["out"]
    return full, res


def kernel(**inputs) -> np.ndarray:
    return run(inputs)[0]
